# revision 21
# baseline (speedup 1.0000x reference)
"""Trainium2 Bass kernel for AttentionLayerPooler.

Computes, for two independent weight/value streams (k and v):
    attn = softmax(logits)                  # [28, 36], tiny -> host
    pooled[m] = sum_l attn[m, l] * x[l]     # [28, B*H*S*D] matmul, device

Sharding: data-parallel over the H axis (16 heads -> 2 heads per core x 8
cores). Each core handles a [36, 262144] slice of ks and vs.

The problem is HBM-bandwidth bound (per-core ~358 GB/s ceiling). Two levers:

1. fp16 on device. The correctness budget (rel err < 2e-2) dwarfs fp16
   quantization (~1e-3), so inputs are cast to fp16 on the host (free), the
   matmul runs fp16 x fp16 -> fp32 PSUM, and outputs are written as fp16 and
   widened on the host. HBM traffic halves: 134 MB -> 67 MB per core.

2. DMA engine balance via host-side repacking. Each SDMA engine owns 8 fixed
   partitions (partitions 0-63 sit on the 8 even engines, 64-127 on the 8
   odd ones), so the natural [36, N] / [28, N] tiles pile onto half the
   engines. Instead the host prepacks, per core and stream:
     xA   [128, NB]    4 column-blocks x layers 0-31 on partition 32j + l
     xBlo [16, NB/2]   layers 32-35, left half of each tile's columns
     xBhi [16, NB/2]   layers 32-35, right half
     out  [112, NB]    4 column-blocks x 28 outputs (partition 28j + m)
   xBlo lands on SBUF partitions 32-47 (even engines), xBhi on 64-79 (odd
   engines) - matmul operands must start at partition 0/32/64, and the
   half-split spreads the remainder bytes over all 16 engines.

Device loop per tile (fw cols x 4 blocks): one 2 MB xA DMA (sync queue),
two 128 KB remainder DMAs (gpsimd), accumulating matmul pairs per 1024-col
slice (K=128 'start' + K=16 'stop') into [112, 2048] PSUM tiles, DVE/ACT
evict with fp32->fp16 cast, one 1.75 MB out-DMA (scalar queue).
"""

import sys

sys.path.insert(0, "/opt/trn_rl_repo")

import numpy as np

import concourse.bass as bass
import concourse.tile as tile
from concourse import bacc, mybir
from concourse.bass_utils import run_bass_kernel_spmd

L, M = 36, 28                   # teacher/student layers
B, H, S, D = 1, 16, 1024, 128
N_CORES = 8
H_PER_CORE = H // N_CORES
NCOLS = H_PER_CORE * S * D      # 262144 columns per core per tensor
PACK = 4                        # column blocks packed on partitions
NB = NCOLS // PACK              # 65536 columns per block
LA, LB = 32, 4                  # layer split: 4*32=128 main + 4*4 remainder
MOUT = PACK * M                 # 112 output partitions
RLO, RHI = 32, 64               # SBUF base partitions for the two R halves

import ml_dtypes

DT16 = "bf16"                   # device 16-bit dtype: "f16" or "bf16"
F16 = mybir.dt.bfloat16 if DT16 == "bf16" else mybir.dt.float16
NP16 = ml_dtypes.bfloat16 if DT16 == "bf16" else np.float16
FP32 = mybir.dt.float32

FW = 8192                       # tile free width
MMW = 512                       # matmul moving free dim (HW per-inst max)
PSW = 2048                      # psum tile width (4 banks)

_NC_CACHE = None


IMPL = 3                        # active implementation: 4 (pack-4) or 3

P3 = 3                          # pack-3 column blocks
NB3 = 86016                     # body columns per block (3*86016 = 258048)
BODY3 = P3 * NB3
TAILC = NCOLS - BODY3           # 4096 tail columns, processed unpacked
MO3 = P3 * M                    # 84 output partitions
FW3 = 12288


def _build_nc(impl=None, **kw):
    if (impl or IMPL) == 3:
        return _build_nc3(**kw)
    return _build_nc4(**kw)


def _build_nc3(reps=1, fw=4096, mmw=MMW, inbufs=6, stbufs=4, psbufs=2,
               psw=PSW, ev=3, altbase=0, mode=0, noev=0, oq=0, iq=0):
    nt = NB3 // fw
    assert nt * fw == NB3 and fw % psw == 0 and psw % mmw == 0

    nc = bacc.Bacc("TRN2", target_bir_lowering=False, debug=False,
                   num_devices=N_CORES)

    kP = nc.dram_tensor("kP", [P3 * L, NB3], F16, kind="ExternalInput")
    kT = nc.dram_tensor("kT", [L, TAILC], F16, kind="ExternalInput")
    vP = nc.dram_tensor("vP", [P3 * L, NB3], F16, kind="ExternalInput")
    vT = nc.dram_tensor("vT", [L, TAILC], F16, kind="ExternalInput")
    w_k3 = nc.dram_tensor("w_k3", [P3 * L, MO3], F16, kind="ExternalInput")
    w_v3 = nc.dram_tensor("w_v3", [P3 * L, MO3], F16, kind="ExternalInput")
    k_out = nc.dram_tensor("k_out", [MO3, NB3], F16, kind="ExternalOutput")
    kt_out = nc.dram_tensor("kt_out", [M, TAILC], F16, kind="ExternalOutput")
    v_out = nc.dram_tensor("v_out", [MO3, NB3], F16, kind="ExternalOutput")
    vt_out = nc.dram_tensor("vt_out", [M, TAILC], F16, kind="ExternalOutput")

    with tile.TileContext(nc) as tc:
        with (
            tc.tile_pool(name="wpool", bufs=1) as wpool,
            tc.tile_pool(name="inpool", bufs=inbufs) as inpool,
            tc.tile_pool(name="tpool", bufs=2) as tpool,
            tc.tile_pool(name="stpool", bufs=stbufs) as stpool,
            tc.tile_pool(name="tspool", bufs=2) as tspool,
            tc.tile_pool(name="pspool", bufs=psbufs, space="PSUM") as pspool,
        ):
            wk = wpool.tile([P3 * L, MO3], F16, tag="wk")
            nc.sync.dma_start(wk[:], w_k3.ap()[:, :])
            wv = wpool.tile([P3 * L, MO3], F16, tag="wv")
            nc.sync.dma_start(wv[:], w_v3.ap()[:, :])

            if mode == 2:
                tin0 = wpool.tile([P3 * L, fw], F16, tag="tin0")
                nc.gpsimd.memset(tin0[:, :], 0.0)

            if reps > 1:
                loop_cm = tc.For_i(0, reps, 1)
                loop_cm.__enter__()

            def evict3(src, dst, idx):
                if noev:
                    return
                if ev == 3:
                    hw = src.shape[-1] // 2
                    nc.vector.tensor_copy(dst[:, 0:hw], src[:, 0:hw])
                    nc.scalar.copy(dst[:, hw:], src[:, hw:])
                elif ev == 1 or (ev == 0 and idx % 2 == 0):
                    nc.vector.tensor_copy(dst[:, :], src[:, :])
                else:
                    nc.scalar.copy(dst[:, :], src[:, :])

            for xp, xt, w, x_out, xt_out in (
                    (kP, kT, wk, k_out, kt_out),
                    (vP, vT, wv, v_out, vt_out)):
                for t in range(nt):
                    b = 32 if (altbase and t % 2) else 0
                    if mode != 2:
                        tin = inpool.tile([P3 * L, fw], F16)
                        if iq == 2:
                            nc.sync.dma_start(
                                tin[0:2 * L, :],
                                xp.ap()[0:2 * L, t * fw:(t + 1) * fw])
                            nc.gpsimd.dma_start(
                                tin[2 * L:3 * L, :],
                                xp.ap()[2 * L:3 * L, t * fw:(t + 1) * fw])
                        else:
                            ie = nc.gpsimd if (iq and t % 2) else nc.sync
                            ie.dma_start(tin[:, :],
                                         xp.ap()[:, t * fw:(t + 1) * fw])
                    else:
                        tin = tin0
                    stage = stpool.tile([32 + MO3, fw], F16)
                    for h in range(fw // psw):
                        ps = pspool.tile([32 + MO3, psw], FP32)
                        for mm in range(psw // mmw):
                            c0 = h * psw + mm * mmw
                            nc.tensor.matmul(
                                ps[b:b + MO3, mm * mmw:(mm + 1) * mmw],
                                w[:, :], tin[:, c0:c0 + mmw],
                                start=True, stop=True)
                        evict3(ps[b:b + MO3, :],
                               stage[b:b + MO3, h * psw:(h + 1) * psw], h)
                    if mode != 2:
                        if oq == 2:
                            nc.scalar.dma_start(
                                x_out.ap()[0:2 * M, t * fw:(t + 1) * fw],
                                stage[b:b + 2 * M, :])
                            nc.gpsimd.dma_start(
                                x_out.ap()[2 * M:3 * M, t * fw:(t + 1) * fw],
                                stage[b + 2 * M:b + 3 * M, :])
                        else:
                            oe = (nc.gpsimd if (oq and t % 2 == 0)
                                  else nc.scalar)
                            oe.dma_start(
                                x_out.ap()[:, t * fw:(t + 1) * fw],
                                stage[b:b + MO3, :])

                # tail: TAILC columns, unpacked (K=36 -> 28 outputs)
                if mode != 2:
                    tint = tpool.tile([L, TAILC], F16)
                    nc.sync.dma_start(tint[:, :], xt.ap()[:, :])
                    staget = tspool.tile([M, TAILC], F16)
                    for h in range(TAILC // psw):
                        ps = pspool.tile([32 + MO3, psw], FP32)
                        for mm in range(psw // mmw):
                            c0 = h * psw + mm * mmw
                            nc.tensor.matmul(
                                ps[0:M, mm * mmw:(mm + 1) * mmw],
                                w[0:L, 0:M], tint[:, c0:c0 + mmw],
                                start=True, stop=True)
                        evict3(ps[0:M, :],
                               staget[:, h * psw:(h + 1) * psw], h)
                    nc.scalar.dma_start(xt_out.ap()[:, :], staget[:, :])

            if reps > 1:
                loop_cm.__exit__(None, None, None)

    nc.compile()
    return nc


def _build_nc4(reps=1, fw=FW, mmw=MMW, inbufs=3, stbufs=3, psbufs=2,
               psw=PSW, bq=0, order=1, ev=0, mode=0, nob=0, noev=0,
               bss=0, evtt=0):
    nt = NB // fw               # tiles per tensor
    fw2 = fw // 2
    assert nt * fw == NB and fw2 % mmw == 0 and fw % psw == 0

    nc = bacc.Bacc("TRN2", target_bir_lowering=False, debug=False,
                   num_devices=N_CORES)

    kA = nc.dram_tensor("kA", [PACK * LA, NB], F16, kind="ExternalInput")
    kLo = nc.dram_tensor("kLo", [PACK * LB, NB // 2], F16, kind="ExternalInput")
    kHi = nc.dram_tensor("kHi", [PACK * LB, NB // 2], F16, kind="ExternalInput")
    vA = nc.dram_tensor("vA", [PACK * LA, NB], F16, kind="ExternalInput")
    vLo = nc.dram_tensor("vLo", [PACK * LB, NB // 2], F16, kind="ExternalInput")
    vHi = nc.dram_tensor("vHi", [PACK * LB, NB // 2], F16, kind="ExternalInput")
    w_ak = nc.dram_tensor("w_ak", [PACK * LA, MOUT], F16, kind="ExternalInput")
    w_bk = nc.dram_tensor("w_bk", [RHI + PACK * LB, MOUT], F16,
                          kind="ExternalInput")
    w_av = nc.dram_tensor("w_av", [PACK * LA, MOUT], F16, kind="ExternalInput")
    w_bv = nc.dram_tensor("w_bv", [RHI + PACK * LB, MOUT], F16,
                          kind="ExternalInput")
    k_out = nc.dram_tensor("k_out", [MOUT, NB], F16, kind="ExternalOutput")
    v_out = nc.dram_tensor("v_out", [MOUT, NB], F16, kind="ExternalOutput")

    with tile.TileContext(nc) as tc:
        with (
            tc.tile_pool(name="wpool", bufs=1) as wpool,
            tc.tile_pool(name="inpool", bufs=inbufs) as inpool,
            tc.tile_pool(name="iblo", bufs=inbufs) as iblo,
            tc.tile_pool(name="ibhi", bufs=inbufs) as ibhi,
            tc.tile_pool(name="stpool", bufs=stbufs) as stpool,
            tc.tile_pool(name="pspool", bufs=psbufs, space="PSUM") as pspool,
        ):
            wak = wpool.tile([PACK * LA, MOUT], F16, tag="wak")
            nc.sync.dma_start(wak[:], w_ak.ap()[:, :])
            wbk = wpool.tile([RHI + PACK * LB, MOUT], F16, tag="wbk")
            nc.sync.dma_start(wbk[:], w_bk.ap()[:, :])
            wav = wpool.tile([PACK * LA, MOUT], F16, tag="wav")
            nc.sync.dma_start(wav[:], w_av.ap()[:, :])
            wbv = wpool.tile([RHI + PACK * LB, MOUT], F16, tag="wbv")
            nc.sync.dma_start(wbv[:], w_bv.ap()[:, :])

            if reps > 1:
                loop_cm = tc.For_i(0, reps, 1)
                loop_cm.__enter__()

            blo_eng, bhi_eng = {
                0: (nc.gpsimd, nc.gpsimd),
                1: (nc.sync, nc.scalar),
                2: (nc.scalar, nc.scalar),
                3: (nc.gpsimd, nc.scalar),
            }[bq]
            if mode == 1:
                # DMA-only: out-DMAs read a fixed prewritten stage
                stage0 = wpool.tile([MOUT, fw], F16, tag="stage0")
                nc.gpsimd.memset(stage0[:, :], 0.0)
            if mode == 2:
                # compute-only: matmuls read fixed input tiles
                tinA0 = wpool.tile([PACK * LA, fw], F16, tag="tinA0")
                nc.gpsimd.memset(tinA0[:, :], 0.0)
                tlo0 = wpool.tile([RLO + PACK * LB, fw2], F16, tag="tlo0")
                nc.gpsimd.memset(tlo0[:, :], 0.0)
                thi0 = wpool.tile([RHI + PACK * LB, fw2], F16, tag="thi0")
                nc.gpsimd.memset(thi0[:, :], 0.0)
            for xa, xlo, xhi, wa, wb, x_out in (
                    (kA, kLo, kHi, wak, wbk, k_out),
                    (vA, vLo, vHi, wav, wbv, v_out)):
                for t in range(nt):
                    if mode != 2:
                        tinA = inpool.tile([PACK * LA, fw], F16)
                        nc.sync.dma_start(tinA[:, :],
                                          xa.ap()[:, t * fw:(t + 1) * fw])
                        tlo = iblo.tile([RLO + PACK * LB, fw2], F16)
                        blo_eng.dma_start(
                            tlo[RLO:RLO + PACK * LB, :],
                            xlo.ap()[:, t * fw2:(t + 1) * fw2])
                        thi = ibhi.tile([RHI + PACK * LB, fw2], F16)
                        bhi_eng.dma_start(
                            thi[RHI:RHI + PACK * LB, :],
                            xhi.ap()[:, t * fw2:(t + 1) * fw2])
                    else:
                        tinA, tlo, thi = tinA0, tlo0, thi0
                    if mode == 1:
                        nc.scalar.dma_start(
                            x_out.ap()[:, t * fw:(t + 1) * fw],
                            stage0[:, :])
                        continue
                    stage = stpool.tile([MOUT, fw], F16)
                    nmm = psw // mmw

                    def _ab(h, mm):
                        c0 = h * psw + mm * mmw
                        if c0 < fw2:
                            return c0, tlo, RLO, c0
                        return c0, thi, RHI, c0 - fw2

                    def mm_a(ps, h, mm):
                        c0, tb, rb, wc = _ab(h, mm)
                        nc.tensor.matmul(
                            ps[:, mm * mmw:(mm + 1) * mmw],
                            wa[:, :], tinA[:, c0:c0 + mmw],
                            start=True, stop=bool(nob))

                    def mm_b(ps, h, mm):
                        if nob:
                            return
                        c0, tb, rb, wc = _ab(h, mm)
                        nc.tensor.matmul(
                            ps[:, mm * mmw:(mm + 1) * mmw],
                            wb[rb:rb + PACK * LB, :],
                            tb[rb:rb + PACK * LB, wc:wc + mmw],
                            start=bool(bss), stop=True)

                    def evict(ps, h, idx):
                        if noev:
                            return
                        c0 = h * psw
                        if evtt:
                            nc.vector.tensor_tensor(
                                stage[:, c0:c0 + psw], ps[:, :], ps[:, :],
                                op=mybir.AluOpType.add)
                            return
                        if ev == 3:
                            hw = psw // 2
                            nc.vector.tensor_copy(
                                stage[:, c0:c0 + hw], ps[:, 0:hw])
                            nc.scalar.copy(
                                stage[:, c0 + hw:c0 + psw], ps[:, hw:psw])
                        elif ev == 1 or (ev == 0 and idx % 2 == 0):
                            nc.vector.tensor_copy(
                                stage[:, c0:c0 + psw], ps[:, :])
                        else:
                            nc.scalar.copy(
                                stage[:, c0:c0 + psw], ps[:, :])

                    if order == 2:
                        for hp in range(0, fw // psw, 2):
                            ps0 = pspool.tile([MOUT, psw], FP32)
                            ps1 = pspool.tile([MOUT, psw], FP32)
                            for mm in range(nmm):
                                mm_a(ps0, hp, mm)
                            for mm in range(nmm):
                                mm_a(ps1, hp + 1, mm)
                            for mm in range(nmm):
                                mm_b(ps0, hp, mm)
                            for mm in range(nmm):
                                mm_b(ps1, hp + 1, mm)
                            evict(ps0, hp, 0)
                            evict(ps1, hp + 1, 1)
                    else:
                        for h in range(fw // psw):
                            ps = pspool.tile([MOUT, psw], FP32)
                            if order == 0:
                                for mm in range(nmm):
                                    mm_a(ps, h, mm)
                                    mm_b(ps, h, mm)
                            else:
                                for mm in range(nmm):
                                    mm_a(ps, h, mm)
                                for mm in range(nmm):
                                    mm_b(ps, h, mm)
                            evict(ps, h, h)
                    if mode != 2:
                        nc.scalar.dma_start(
                            x_out.ap()[:, t * fw:(t + 1) * fw], stage[:, :])

            if reps > 1:
                loop_cm.__exit__(None, None, None)

    nc.compile()
    return nc


def _get_nc():
    global _NC_CACHE
    if _NC_CACHE is None:
        _NC_CACHE = _build_nc()
    return _NC_CACHE


def _softmax_f32(x):
    x = np.asarray(x, np.float32)
    x = x - x.max(axis=-1, keepdims=True)
    e = np.exp(x)
    return (e / e.sum(axis=-1, keepdims=True)).astype(np.float32)


def _weights(attn):
    # wA[32j + l, 28j + m] = attn[m, l]            (l < 32)
    # wB[rb + 4j + l', 28j + m] = attn[m, 32 + l'] for rb in (RLO, RHI)
    wt = np.ascontiguousarray(attn.T).astype(NP16)  # [36, 28]
    wA = np.zeros((PACK * LA, MOUT), NP16)
    wB = np.zeros((RHI + PACK * LB, MOUT), NP16)
    for j in range(PACK):
        wA[LA * j:LA * j + LA, M * j:M * j + M] = wt[:LA]
        for rb in (RLO, RHI):
            wB[rb + LB * j:rb + LB * j + LB, M * j:M * j + M] = wt[LA:]
    return wA, wB


def _pack_x(x16, fw):
    # x16: [36, NCOLS] fp16 -> (xA [128, NB], xBlo/xBhi [16, NB/2])
    nt = NB // fw
    fw2 = fw // 2
    x4 = x16.reshape(L, PACK, NB)
    xA = np.ascontiguousarray(
        x4[:LA].transpose(1, 0, 2)).reshape(PACK * LA, NB)
    xr = x4[LA:].reshape(LB, PACK, nt, 2, fw2)        # (l', j, t, half, u)
    xlo = np.ascontiguousarray(
        xr[:, :, :, 0].transpose(1, 0, 2, 3)).reshape(PACK * LB, nt * fw2)
    xhi = np.ascontiguousarray(
        xr[:, :, :, 1].transpose(1, 0, 2, 3)).reshape(PACK * LB, nt * fw2)
    return xA, xlo, xhi


def _weights3(attn):
    # w3[36j + l, 28j + m] = attn[m, l]
    wt = np.ascontiguousarray(attn.T).astype(NP16)  # [36, 28]
    w3 = np.zeros((P3 * L, MO3), NP16)
    for j in range(P3):
        w3[L * j:L * j + L, M * j:M * j + M] = wt
    return w3


def _pack_x3(x16):
    # x16: [36, NCOLS] -> (xP [108, NB3], xT [36, TAILC])
    xb = x16[:, :BODY3].reshape(L, P3, NB3)
    xP = np.ascontiguousarray(xb.transpose(1, 0, 2)).reshape(P3 * L, NB3)
    xT = np.ascontiguousarray(x16[:, BODY3:])
    return xP, xT


def make_core_inputs(ks, vs, attn_logits_k, attn_logits_v, fw=None,
                     impl=None):
    """Host-side prep: per-core input dicts (16-bit, DMA-balanced layout)."""
    impl = impl or IMPL
    ks = np.asarray(ks, np.float32)
    vs = np.asarray(vs, np.float32)
    if impl == 3:
        wk3 = _weights3(_softmax_f32(attn_logits_k))
        wv3 = _weights3(_softmax_f32(attn_logits_v))
    else:
        wak, wbk = _weights(_softmax_f32(attn_logits_k))
        wav, wbv = _weights(_softmax_f32(attn_logits_v))
    maps = []
    for c in range(N_CORES):
        h0 = c * H_PER_CORE
        k16 = np.ascontiguousarray(
            ks[:, 0, h0:h0 + H_PER_CORE]).reshape(L, NCOLS).astype(NP16)
        v16 = np.ascontiguousarray(
            vs[:, 0, h0:h0 + H_PER_CORE]).reshape(L, NCOLS).astype(NP16)
        if impl == 3:
            kPh, kTh = _pack_x3(k16)
            vPh, vTh = _pack_x3(v16)
            maps.append({
                "kP": kPh, "kT": kTh, "vP": vPh, "vT": vTh,
                "w_k3": wk3, "w_v3": wv3,
            })
        else:
            kAh, kLoh, kHih = _pack_x(k16, fw or FW)
            vAh, vLoh, vHih = _pack_x(v16, fw or FW)
            maps.append({
                "kA": kAh, "kLo": kLoh, "kHi": kHih,
                "vA": vAh, "vLo": vLoh, "vHi": vHih,
                "w_ak": wak, "w_bk": wbk, "w_av": wav, "w_bv": wbv,
            })
    return maps


def _unpack_out(y16):
    # [112, NB] fp16 -> [28, H_PER_CORE, S, D] fp32
    return (y16.reshape(PACK, M, NB).transpose(1, 0, 2)
            .reshape(M, H_PER_CORE, S, D).astype(np.float32))


def _unpack_out3(yP, yT):
    # [84, NB3] + [28, TAILC] -> [28, H_PER_CORE, S, D] fp32
    y = np.empty((M, NCOLS), np.float32)
    y[:, :BODY3] = (yP.reshape(P3, M, NB3).transpose(1, 0, 2)
                    .reshape(M, BODY3).astype(np.float32))
    y[:, BODY3:] = yT.astype(np.float32)
    return y.reshape(M, H_PER_CORE, S, D)


def kernel(ks, vs, attn_logits_k, attn_logits_v, _trace=False,
           _trace_kwargs=None):
    nc = _get_nc()
    in_maps = make_core_inputs(ks, vs, attn_logits_k, attn_logits_v)

    res = run_bass_kernel_spmd(
        nc, in_maps, core_ids=list(range(N_CORES)),
        trace=_trace, **(_trace_kwargs or {}),
    )

    ks_pooled = np.empty((M, B, H, S, D), np.float32)
    vs_pooled = np.empty((M, B, H, S, D), np.float32)
    for c in range(N_CORES):
        h0 = c * H_PER_CORE
        r = res.results[c]
        if IMPL == 3:
            ks_pooled[:, 0, h0:h0 + H_PER_CORE] = _unpack_out3(
                r["k_out"], r["kt_out"])
            vs_pooled[:, 0, h0:h0 + H_PER_CORE] = _unpack_out3(
                r["v_out"], r["vt_out"])
        else:
            ks_pooled[:, 0, h0:h0 + H_PER_CORE] = _unpack_out(r["k_out"])
            vs_pooled[:, 0, h0:h0 + H_PER_CORE] = _unpack_out(r["v_out"])

    if _trace:
        return (ks_pooled, vs_pooled), res
    return (ks_pooled, vs_pooled)


# revision 23
# speedup vs baseline: 1.0043x; 1.0043x over previous
"""Trainium2 Bass kernel for AttentionLayerPooler.

Computes, for two independent weight/value streams (k and v):
    attn = softmax(logits)                  # [28, 36], tiny -> host
    pooled[m] = sum_l attn[m, l] * x[l]     # [28, B*H*S*D] matmul, device

Sharding: data-parallel over the H axis (16 heads -> 2 heads per core x 8
cores). Each core handles a [36, 262144] slice of ks and vs.

The problem is HBM-bandwidth bound (per-core ~358 GB/s ceiling). Two levers:

1. fp16 on device. The correctness budget (rel err < 2e-2) dwarfs fp16
   quantization (~1e-3), so inputs are cast to fp16 on the host (free), the
   matmul runs fp16 x fp16 -> fp32 PSUM, and outputs are written as fp16 and
   widened on the host. HBM traffic halves: 134 MB -> 67 MB per core.

2. DMA engine balance via host-side repacking. Each SDMA engine owns 8 fixed
   partitions (partitions 0-63 sit on the 8 even engines, 64-127 on the 8
   odd ones), so the natural [36, N] / [28, N] tiles pile onto half the
   engines. Instead the host prepacks, per core and stream:
     xA   [128, NB]    4 column-blocks x layers 0-31 on partition 32j + l
     xBlo [16, NB/2]   layers 32-35, left half of each tile's columns
     xBhi [16, NB/2]   layers 32-35, right half
     out  [112, NB]    4 column-blocks x 28 outputs (partition 28j + m)
   xBlo lands on SBUF partitions 32-47 (even engines), xBhi on 64-79 (odd
   engines) - matmul operands must start at partition 0/32/64, and the
   half-split spreads the remainder bytes over all 16 engines.

Device loop per tile (fw cols x 4 blocks): one 2 MB xA DMA (sync queue),
two 128 KB remainder DMAs (gpsimd), accumulating matmul pairs per 1024-col
slice (K=128 'start' + K=16 'stop') into [112, 2048] PSUM tiles, DVE/ACT
evict with fp32->fp16 cast, one 1.75 MB out-DMA (scalar queue).
"""

import sys

sys.path.insert(0, "/opt/trn_rl_repo")

import numpy as np

import concourse.bass as bass
import concourse.tile as tile
from concourse import bacc, mybir
from concourse.bass_utils import run_bass_kernel_spmd

L, M = 36, 28                   # teacher/student layers
B, H, S, D = 1, 16, 1024, 128
N_CORES = 8
H_PER_CORE = H // N_CORES
NCOLS = H_PER_CORE * S * D      # 262144 columns per core per tensor
PACK = 4                        # column blocks packed on partitions
NB = NCOLS // PACK              # 65536 columns per block
LA, LB = 32, 4                  # layer split: 4*32=128 main + 4*4 remainder
MOUT = PACK * M                 # 112 output partitions
RLO, RHI = 32, 64               # SBUF base partitions for the two R halves

import ml_dtypes

DT16 = "bf16"                   # device 16-bit dtype: "f16" or "bf16"
F16 = mybir.dt.bfloat16 if DT16 == "bf16" else mybir.dt.float16
NP16 = ml_dtypes.bfloat16 if DT16 == "bf16" else np.float16
FP32 = mybir.dt.float32

FW = 8192                       # tile free width
MMW = 512                       # matmul moving free dim (HW per-inst max)
PSW = 2048                      # psum tile width (4 banks)

_NC_CACHE = None


IMPL = 3                        # active implementation: 4 (pack-4) or 3

P3 = 3                          # pack-3 column blocks
NB3 = 86016                     # body columns per block (3*86016 = 258048)
BODY3 = P3 * NB3
TAILC = NCOLS - BODY3           # 4096 tail columns, processed unpacked
MO3 = P3 * M                    # 84 output partitions
FW3 = 12288


def _build_nc(impl=None, **kw):
    if (impl or IMPL) == 3:
        return _build_nc3(**kw)
    return _build_nc4(**kw)


def _build_nc3(reps=1, fw=4096, mmw=MMW, inbufs=6, stbufs=4, psbufs=2,
               psw=PSW, ev=3, altbase=0, mode=0, noev=0, oq=0, iq=1):
    nt = NB3 // fw
    assert nt * fw == NB3 and fw % psw == 0 and psw % mmw == 0

    nc = bacc.Bacc("TRN2", target_bir_lowering=False, debug=False,
                   num_devices=N_CORES)

    kP = nc.dram_tensor("kP", [P3 * L, NB3], F16, kind="ExternalInput")
    kT = nc.dram_tensor("kT", [L, TAILC], F16, kind="ExternalInput")
    vP = nc.dram_tensor("vP", [P3 * L, NB3], F16, kind="ExternalInput")
    vT = nc.dram_tensor("vT", [L, TAILC], F16, kind="ExternalInput")
    w_k3 = nc.dram_tensor("w_k3", [P3 * L, MO3], F16, kind="ExternalInput")
    w_v3 = nc.dram_tensor("w_v3", [P3 * L, MO3], F16, kind="ExternalInput")
    k_out = nc.dram_tensor("k_out", [MO3, NB3], F16, kind="ExternalOutput")
    kt_out = nc.dram_tensor("kt_out", [M, TAILC], F16, kind="ExternalOutput")
    v_out = nc.dram_tensor("v_out", [MO3, NB3], F16, kind="ExternalOutput")
    vt_out = nc.dram_tensor("vt_out", [M, TAILC], F16, kind="ExternalOutput")

    with tile.TileContext(nc) as tc:
        with (
            tc.tile_pool(name="wpool", bufs=1) as wpool,
            tc.tile_pool(name="inpool", bufs=inbufs) as inpool,
            tc.tile_pool(name="tpool", bufs=2) as tpool,
            tc.tile_pool(name="stpool", bufs=stbufs) as stpool,
            tc.tile_pool(name="tspool", bufs=2) as tspool,
            tc.tile_pool(name="pspool", bufs=psbufs, space="PSUM") as pspool,
        ):
            wk = wpool.tile([P3 * L, MO3], F16, tag="wk")
            nc.sync.dma_start(wk[:], w_k3.ap()[:, :])
            wv = wpool.tile([P3 * L, MO3], F16, tag="wv")
            nc.sync.dma_start(wv[:], w_v3.ap()[:, :])

            if mode == 2:
                tin0 = wpool.tile([P3 * L, fw], F16, tag="tin0")
                nc.gpsimd.memset(tin0[:, :], 0.0)

            if reps > 1:
                loop_cm = tc.For_i(0, reps, 1)
                loop_cm.__enter__()

            def evict3(src, dst, idx):
                if noev:
                    return
                if ev == 3:
                    hw = src.shape[-1] // 2
                    nc.vector.tensor_copy(dst[:, 0:hw], src[:, 0:hw])
                    nc.scalar.copy(dst[:, hw:], src[:, hw:])
                elif ev == 1 or (ev == 0 and idx % 2 == 0):
                    nc.vector.tensor_copy(dst[:, :], src[:, :])
                else:
                    nc.scalar.copy(dst[:, :], src[:, :])

            for xp, xt, w, x_out, xt_out in (
                    (kP, kT, wk, k_out, kt_out),
                    (vP, vT, wv, v_out, vt_out)):
                for t in range(nt):
                    b = 32 if (altbase and t % 2) else 0
                    if mode != 2:
                        tin = inpool.tile([P3 * L, fw], F16)
                        if iq == 2:
                            nc.sync.dma_start(
                                tin[0:2 * L, :],
                                xp.ap()[0:2 * L, t * fw:(t + 1) * fw])
                            nc.gpsimd.dma_start(
                                tin[2 * L:3 * L, :],
                                xp.ap()[2 * L:3 * L, t * fw:(t + 1) * fw])
                        else:
                            ie = nc.gpsimd if (iq and t % 2) else nc.sync
                            ie.dma_start(tin[:, :],
                                         xp.ap()[:, t * fw:(t + 1) * fw])
                    else:
                        tin = tin0
                    stage = stpool.tile([32 + MO3, fw], F16)
                    for h in range(fw // psw):
                        ps = pspool.tile([32 + MO3, psw], FP32)
                        for mm in range(psw // mmw):
                            c0 = h * psw + mm * mmw
                            nc.tensor.matmul(
                                ps[b:b + MO3, mm * mmw:(mm + 1) * mmw],
                                w[:, :], tin[:, c0:c0 + mmw],
                                start=True, stop=True)
                        evict3(ps[b:b + MO3, :],
                               stage[b:b + MO3, h * psw:(h + 1) * psw], h)
                    if mode != 2:
                        if oq == 2:
                            nc.scalar.dma_start(
                                x_out.ap()[0:2 * M, t * fw:(t + 1) * fw],
                                stage[b:b + 2 * M, :])
                            nc.gpsimd.dma_start(
                                x_out.ap()[2 * M:3 * M, t * fw:(t + 1) * fw],
                                stage[b + 2 * M:b + 3 * M, :])
                        else:
                            oe = nc.scalar
                            if oq == 3:
                                oe = nc.gpsimd
                            elif oq == 1 and t % 2 == 0:
                                oe = nc.gpsimd
                            oe.dma_start(
                                x_out.ap()[:, t * fw:(t + 1) * fw],
                                stage[b:b + MO3, :])

                # tail: TAILC columns, unpacked (K=36 -> 28 outputs)
                if mode != 2:
                    tint = tpool.tile([L, TAILC], F16)
                    nc.sync.dma_start(tint[:, :], xt.ap()[:, :])
                    staget = tspool.tile([M, TAILC], F16)
                    for h in range(TAILC // psw):
                        ps = pspool.tile([32 + MO3, psw], FP32)
                        for mm in range(psw // mmw):
                            c0 = h * psw + mm * mmw
                            nc.tensor.matmul(
                                ps[0:M, mm * mmw:(mm + 1) * mmw],
                                w[0:L, 0:M], tint[:, c0:c0 + mmw],
                                start=True, stop=True)
                        evict3(ps[0:M, :],
                               staget[:, h * psw:(h + 1) * psw], h)
                    nc.scalar.dma_start(xt_out.ap()[:, :], staget[:, :])

            if reps > 1:
                loop_cm.__exit__(None, None, None)

    nc.compile()
    return nc


def _build_nc4(reps=1, fw=FW, mmw=MMW, inbufs=3, stbufs=3, psbufs=2,
               psw=PSW, bq=0, order=1, ev=0, mode=0, nob=0, noev=0,
               bss=0, evtt=0):
    nt = NB // fw               # tiles per tensor
    fw2 = fw // 2
    assert nt * fw == NB and fw2 % mmw == 0 and fw % psw == 0

    nc = bacc.Bacc("TRN2", target_bir_lowering=False, debug=False,
                   num_devices=N_CORES)

    kA = nc.dram_tensor("kA", [PACK * LA, NB], F16, kind="ExternalInput")
    kLo = nc.dram_tensor("kLo", [PACK * LB, NB // 2], F16, kind="ExternalInput")
    kHi = nc.dram_tensor("kHi", [PACK * LB, NB // 2], F16, kind="ExternalInput")
    vA = nc.dram_tensor("vA", [PACK * LA, NB], F16, kind="ExternalInput")
    vLo = nc.dram_tensor("vLo", [PACK * LB, NB // 2], F16, kind="ExternalInput")
    vHi = nc.dram_tensor("vHi", [PACK * LB, NB // 2], F16, kind="ExternalInput")
    w_ak = nc.dram_tensor("w_ak", [PACK * LA, MOUT], F16, kind="ExternalInput")
    w_bk = nc.dram_tensor("w_bk", [RHI + PACK * LB, MOUT], F16,
                          kind="ExternalInput")
    w_av = nc.dram_tensor("w_av", [PACK * LA, MOUT], F16, kind="ExternalInput")
    w_bv = nc.dram_tensor("w_bv", [RHI + PACK * LB, MOUT], F16,
                          kind="ExternalInput")
    k_out = nc.dram_tensor("k_out", [MOUT, NB], F16, kind="ExternalOutput")
    v_out = nc.dram_tensor("v_out", [MOUT, NB], F16, kind="ExternalOutput")

    with tile.TileContext(nc) as tc:
        with (
            tc.tile_pool(name="wpool", bufs=1) as wpool,
            tc.tile_pool(name="inpool", bufs=inbufs) as inpool,
            tc.tile_pool(name="iblo", bufs=inbufs) as iblo,
            tc.tile_pool(name="ibhi", bufs=inbufs) as ibhi,
            tc.tile_pool(name="stpool", bufs=stbufs) as stpool,
            tc.tile_pool(name="pspool", bufs=psbufs, space="PSUM") as pspool,
        ):
            wak = wpool.tile([PACK * LA, MOUT], F16, tag="wak")
            nc.sync.dma_start(wak[:], w_ak.ap()[:, :])
            wbk = wpool.tile([RHI + PACK * LB, MOUT], F16, tag="wbk")
            nc.sync.dma_start(wbk[:], w_bk.ap()[:, :])
            wav = wpool.tile([PACK * LA, MOUT], F16, tag="wav")
            nc.sync.dma_start(wav[:], w_av.ap()[:, :])
            wbv = wpool.tile([RHI + PACK * LB, MOUT], F16, tag="wbv")
            nc.sync.dma_start(wbv[:], w_bv.ap()[:, :])

            if reps > 1:
                loop_cm = tc.For_i(0, reps, 1)
                loop_cm.__enter__()

            blo_eng, bhi_eng = {
                0: (nc.gpsimd, nc.gpsimd),
                1: (nc.sync, nc.scalar),
                2: (nc.scalar, nc.scalar),
                3: (nc.gpsimd, nc.scalar),
            }[bq]
            if mode == 1:
                # DMA-only: out-DMAs read a fixed prewritten stage
                stage0 = wpool.tile([MOUT, fw], F16, tag="stage0")
                nc.gpsimd.memset(stage0[:, :], 0.0)
            if mode == 2:
                # compute-only: matmuls read fixed input tiles
                tinA0 = wpool.tile([PACK * LA, fw], F16, tag="tinA0")
                nc.gpsimd.memset(tinA0[:, :], 0.0)
                tlo0 = wpool.tile([RLO + PACK * LB, fw2], F16, tag="tlo0")
                nc.gpsimd.memset(tlo0[:, :], 0.0)
                thi0 = wpool.tile([RHI + PACK * LB, fw2], F16, tag="thi0")
                nc.gpsimd.memset(thi0[:, :], 0.0)
            for xa, xlo, xhi, wa, wb, x_out in (
                    (kA, kLo, kHi, wak, wbk, k_out),
                    (vA, vLo, vHi, wav, wbv, v_out)):
                for t in range(nt):
                    if mode != 2:
                        tinA = inpool.tile([PACK * LA, fw], F16)
                        nc.sync.dma_start(tinA[:, :],
                                          xa.ap()[:, t * fw:(t + 1) * fw])
                        tlo = iblo.tile([RLO + PACK * LB, fw2], F16)
                        blo_eng.dma_start(
                            tlo[RLO:RLO + PACK * LB, :],
                            xlo.ap()[:, t * fw2:(t + 1) * fw2])
                        thi = ibhi.tile([RHI + PACK * LB, fw2], F16)
                        bhi_eng.dma_start(
                            thi[RHI:RHI + PACK * LB, :],
                            xhi.ap()[:, t * fw2:(t + 1) * fw2])
                    else:
                        tinA, tlo, thi = tinA0, tlo0, thi0
                    if mode == 1:
                        nc.scalar.dma_start(
                            x_out.ap()[:, t * fw:(t + 1) * fw],
                            stage0[:, :])
                        continue
                    stage = stpool.tile([MOUT, fw], F16)
                    nmm = psw // mmw

                    def _ab(h, mm):
                        c0 = h * psw + mm * mmw
                        if c0 < fw2:
                            return c0, tlo, RLO, c0
                        return c0, thi, RHI, c0 - fw2

                    def mm_a(ps, h, mm):
                        c0, tb, rb, wc = _ab(h, mm)
                        nc.tensor.matmul(
                            ps[:, mm * mmw:(mm + 1) * mmw],
                            wa[:, :], tinA[:, c0:c0 + mmw],
                            start=True, stop=bool(nob))

                    def mm_b(ps, h, mm):
                        if nob:
                            return
                        c0, tb, rb, wc = _ab(h, mm)
                        nc.tensor.matmul(
                            ps[:, mm * mmw:(mm + 1) * mmw],
                            wb[rb:rb + PACK * LB, :],
                            tb[rb:rb + PACK * LB, wc:wc + mmw],
                            start=bool(bss), stop=True)

                    def evict(ps, h, idx):
                        if noev:
                            return
                        c0 = h * psw
                        if evtt:
                            nc.vector.tensor_tensor(
                                stage[:, c0:c0 + psw], ps[:, :], ps[:, :],
                                op=mybir.AluOpType.add)
                            return
                        if ev == 3:
                            hw = psw // 2
                            nc.vector.tensor_copy(
                                stage[:, c0:c0 + hw], ps[:, 0:hw])
                            nc.scalar.copy(
                                stage[:, c0 + hw:c0 + psw], ps[:, hw:psw])
                        elif ev == 1 or (ev == 0 and idx % 2 == 0):
                            nc.vector.tensor_copy(
                                stage[:, c0:c0 + psw], ps[:, :])
                        else:
                            nc.scalar.copy(
                                stage[:, c0:c0 + psw], ps[:, :])

                    if order == 2:
                        for hp in range(0, fw // psw, 2):
                            ps0 = pspool.tile([MOUT, psw], FP32)
                            ps1 = pspool.tile([MOUT, psw], FP32)
                            for mm in range(nmm):
                                mm_a(ps0, hp, mm)
                            for mm in range(nmm):
                                mm_a(ps1, hp + 1, mm)
                            for mm in range(nmm):
                                mm_b(ps0, hp, mm)
                            for mm in range(nmm):
                                mm_b(ps1, hp + 1, mm)
                            evict(ps0, hp, 0)
                            evict(ps1, hp + 1, 1)
                    else:
                        for h in range(fw // psw):
                            ps = pspool.tile([MOUT, psw], FP32)
                            if order == 0:
                                for mm in range(nmm):
                                    mm_a(ps, h, mm)
                                    mm_b(ps, h, mm)
                            else:
                                for mm in range(nmm):
                                    mm_a(ps, h, mm)
                                for mm in range(nmm):
                                    mm_b(ps, h, mm)
                            evict(ps, h, h)
                    if mode != 2:
                        nc.scalar.dma_start(
                            x_out.ap()[:, t * fw:(t + 1) * fw], stage[:, :])

            if reps > 1:
                loop_cm.__exit__(None, None, None)

    nc.compile()
    return nc


def _get_nc():
    global _NC_CACHE
    if _NC_CACHE is None:
        _NC_CACHE = _build_nc()
    return _NC_CACHE


def _softmax_f32(x):
    x = np.asarray(x, np.float32)
    x = x - x.max(axis=-1, keepdims=True)
    e = np.exp(x)
    return (e / e.sum(axis=-1, keepdims=True)).astype(np.float32)


def _weights(attn):
    # wA[32j + l, 28j + m] = attn[m, l]            (l < 32)
    # wB[rb + 4j + l', 28j + m] = attn[m, 32 + l'] for rb in (RLO, RHI)
    wt = np.ascontiguousarray(attn.T).astype(NP16)  # [36, 28]
    wA = np.zeros((PACK * LA, MOUT), NP16)
    wB = np.zeros((RHI + PACK * LB, MOUT), NP16)
    for j in range(PACK):
        wA[LA * j:LA * j + LA, M * j:M * j + M] = wt[:LA]
        for rb in (RLO, RHI):
            wB[rb + LB * j:rb + LB * j + LB, M * j:M * j + M] = wt[LA:]
    return wA, wB


def _pack_x(x16, fw):
    # x16: [36, NCOLS] fp16 -> (xA [128, NB], xBlo/xBhi [16, NB/2])
    nt = NB // fw
    fw2 = fw // 2
    x4 = x16.reshape(L, PACK, NB)
    xA = np.ascontiguousarray(
        x4[:LA].transpose(1, 0, 2)).reshape(PACK * LA, NB)
    xr = x4[LA:].reshape(LB, PACK, nt, 2, fw2)        # (l', j, t, half, u)
    xlo = np.ascontiguousarray(
        xr[:, :, :, 0].transpose(1, 0, 2, 3)).reshape(PACK * LB, nt * fw2)
    xhi = np.ascontiguousarray(
        xr[:, :, :, 1].transpose(1, 0, 2, 3)).reshape(PACK * LB, nt * fw2)
    return xA, xlo, xhi


def _weights3(attn):
    # w3[36j + l, 28j + m] = attn[m, l]
    wt = np.ascontiguousarray(attn.T).astype(NP16)  # [36, 28]
    w3 = np.zeros((P3 * L, MO3), NP16)
    for j in range(P3):
        w3[L * j:L * j + L, M * j:M * j + M] = wt
    return w3


def _pack_x3(x16):
    # x16: [36, NCOLS] -> (xP [108, NB3], xT [36, TAILC])
    xb = x16[:, :BODY3].reshape(L, P3, NB3)
    xP = np.ascontiguousarray(xb.transpose(1, 0, 2)).reshape(P3 * L, NB3)
    xT = np.ascontiguousarray(x16[:, BODY3:])
    return xP, xT


def make_core_inputs(ks, vs, attn_logits_k, attn_logits_v, fw=None,
                     impl=None):
    """Host-side prep: per-core input dicts (16-bit, DMA-balanced layout)."""
    impl = impl or IMPL
    ks = np.asarray(ks, np.float32)
    vs = np.asarray(vs, np.float32)
    if impl == 3:
        wk3 = _weights3(_softmax_f32(attn_logits_k))
        wv3 = _weights3(_softmax_f32(attn_logits_v))
    else:
        wak, wbk = _weights(_softmax_f32(attn_logits_k))
        wav, wbv = _weights(_softmax_f32(attn_logits_v))
    maps = []
    for c in range(N_CORES):
        h0 = c * H_PER_CORE
        k16 = np.ascontiguousarray(
            ks[:, 0, h0:h0 + H_PER_CORE]).reshape(L, NCOLS).astype(NP16)
        v16 = np.ascontiguousarray(
            vs[:, 0, h0:h0 + H_PER_CORE]).reshape(L, NCOLS).astype(NP16)
        if impl == 3:
            kPh, kTh = _pack_x3(k16)
            vPh, vTh = _pack_x3(v16)
            maps.append({
                "kP": kPh, "kT": kTh, "vP": vPh, "vT": vTh,
                "w_k3": wk3, "w_v3": wv3,
            })
        else:
            kAh, kLoh, kHih = _pack_x(k16, fw or FW)
            vAh, vLoh, vHih = _pack_x(v16, fw or FW)
            maps.append({
                "kA": kAh, "kLo": kLoh, "kHi": kHih,
                "vA": vAh, "vLo": vLoh, "vHi": vHih,
                "w_ak": wak, "w_bk": wbk, "w_av": wav, "w_bv": wbv,
            })
    return maps


def _unpack_out(y16):
    # [112, NB] fp16 -> [28, H_PER_CORE, S, D] fp32
    return (y16.reshape(PACK, M, NB).transpose(1, 0, 2)
            .reshape(M, H_PER_CORE, S, D).astype(np.float32))


def _unpack_out3(yP, yT):
    # [84, NB3] + [28, TAILC] -> [28, H_PER_CORE, S, D] fp32
    y = np.empty((M, NCOLS), np.float32)
    y[:, :BODY3] = (yP.reshape(P3, M, NB3).transpose(1, 0, 2)
                    .reshape(M, BODY3).astype(np.float32))
    y[:, BODY3:] = yT.astype(np.float32)
    return y.reshape(M, H_PER_CORE, S, D)


def kernel(ks, vs, attn_logits_k, attn_logits_v, _trace=False,
           _trace_kwargs=None):
    nc = _get_nc()
    in_maps = make_core_inputs(ks, vs, attn_logits_k, attn_logits_v)

    res = run_bass_kernel_spmd(
        nc, in_maps, core_ids=list(range(N_CORES)),
        trace=_trace, **(_trace_kwargs or {}),
    )

    ks_pooled = np.empty((M, B, H, S, D), np.float32)
    vs_pooled = np.empty((M, B, H, S, D), np.float32)
    for c in range(N_CORES):
        h0 = c * H_PER_CORE
        r = res.results[c]
        if IMPL == 3:
            ks_pooled[:, 0, h0:h0 + H_PER_CORE] = _unpack_out3(
                r["k_out"], r["kt_out"])
            vs_pooled[:, 0, h0:h0 + H_PER_CORE] = _unpack_out3(
                r["v_out"], r["vt_out"])
        else:
            ks_pooled[:, 0, h0:h0 + H_PER_CORE] = _unpack_out(r["k_out"])
            vs_pooled[:, 0, h0:h0 + H_PER_CORE] = _unpack_out(r["v_out"])

    if _trace:
        return (ks_pooled, vs_pooled), res
    return (ks_pooled, vs_pooled)
